# revision 2
# baseline (speedup 1.0000x reference)
"""Trainium2 Bass kernel for nn_AttnMap: out = relu(einsum(dec,enc) @ W + bias).

Math: scores[b,t,hw,(q,g)] = sum_c dec[b,g,q,t,c] * enc[b,t,hw,(g,c)]
      out = relu(scores @ W + bias)
Fusion: out[b,t] = relu(enc[b,t] @ M_t + bias) with
      M_t[(g,c), f] = sum_q dec[b,q,t,(g,c)] * W[q*8+g, f]   ([256,256] per t)

Sharding: data-parallel over batch b across the 8 NeuronCores.

Device kernel (per core, per t = 0..15), all-bf16 data path:
  enc arrives HOST-pre-transposed as encT[t, C, hw] bf16 and the output is
  produced transposed (outT[t, F, hw] bf16, un-transposed on host), so the
  device does zero transposes and half the f32 HBM traffic:
  1. DMA encT 2-t chunks [128, 4096] bf16 -> SBUF (C-chunk on partitions).
  2. M_t via 8 bf16 matmuls (K=16 over q, col-group-tiled) -> PSUM
     -> bf16 SBUF (DVE).
  3. outT chunks po[f128, hw512] via 2 accumulating bf16 matmuls
     (M_t chunk stationary [C128, f128], encT moving [C128, hw512])
     -> relu-evac split ACT/DVE -> bf16 SBUF -> DMA out.
"""
import numpy as np
from contextlib import ExitStack

B, T, HW, C, F = 8, 16, 1024, 256, 256
G, CG, Q = 8, 32, 16  # heads, head dim, queries

_cache = {}


def _build(with_bias: bool, reps: int = 1, tune: dict | None = None):
    import concourse.tile as tile
    from concourse import bacc, mybir

    tune = dict(tune or {})
    DMA_T = tune.get("dma_t", 2)        # t's per enc/out DMA
    BUFS_ENC = tune.get("bufs_enc", 3)
    BUFS_OUT = tune.get("bufs_out", 3)
    BUFS_PO = tune.get("bufs_po", 6)
    BUFS_PM = tune.get("bufs_pm", 2)
    EVAC = tune.get("evac", "split")    # po evac: split|act|dve
    M_EVAC = tune.get("m_evac", "dve")  # pm evac: dve|act
    MODE = tune.get("mode", "full")     # full|dma_only

    f32 = mybir.dt.float32
    bf16 = mybir.dt.bfloat16
    Relu = mybir.ActivationFunctionType.Relu

    nc = bacc.Bacc("TRN2", target_bir_lowering=False, debug=False,
                   num_devices=8)

    t_encT = nc.dram_tensor("encT", [T, C, HW], bf16,
                            kind="ExternalInput").ap()
    t_dq = nc.dram_tensor("dq", [Q, T * C], bf16, kind="ExternalInput").ap()
    t_wp = nc.dram_tensor("wp", [Q, G * F], bf16, kind="ExternalInput").ap()
    if with_bias:
        t_bias = nc.dram_tensor("biasT", [1, F], bf16,
                                kind="ExternalInput").ap()
    t_out = nc.dram_tensor("outT", [T, F, HW], bf16,
                           kind="ExternalOutput").ap()

    with tile.TileContext(nc) as tc, ExitStack() as ctx:
        const = ctx.enter_context(tc.tile_pool(name="const", bufs=1))
        encp = ctx.enter_context(tc.tile_pool(name="encp", bufs=BUFS_ENC))
        outsp = ctx.enter_context(tc.tile_pool(name="outsp", bufs=BUFS_OUT))
        mp = ctx.enter_context(tc.tile_pool(name="mp", bufs=2))
        ps_m = ctx.enter_context(tc.tile_pool(name="ps_m", bufs=BUFS_PM,
                                              space="PSUM"))
        ps_o = ctx.enter_context(tc.tile_pool(name="ps_o", bufs=BUFS_PO,
                                              space="PSUM"))

        # dec as [q, (t, g, c)] bf16, W permuted on host to [q, (g, f)] bf16
        s_dq = const.tile([Q, T * C], bf16, tag="dq")
        nc.sync.dma_start(s_dq[:], t_dq)
        s_wp = const.tile([Q, G * F], bf16, tag="wp")
        nc.sync.dma_start(s_wp[:], t_wp)
        if with_bias:
            s_ones = const.tile([1, 512], bf16, tag="ones")
            nc.gpsimd.memset(s_ones[:], 1.0)
            s_bias = const.tile([1, F], bf16, tag="bias")
            nc.sync.dma_start(s_bias[:], t_bias)

        rep_loop = (tc.For_i(0, reps, 1,
                             hint_engines=(mybir.EngineType.PE,
                                           mybir.EngineType.DVE,
                                           mybir.EngineType.Activation,
                                           mybir.EngineType.SP))
                    if reps > 1 else None)
        if rep_loop is not None:
            ctx.enter_context(rep_loop)

        evac_k = 0
        for tg in range(T // DMA_T):
            # encT load: sbuf[p, (tl, ch, hw)] = encT[t, ch*128+p, hw]
            enc_sb = encp.tile([128, 2048 * DMA_T], bf16, tag="enc")
            nc.sync.dma_start(
                enc_sb[:].rearrange("p (t ch hw) -> p t ch hw",
                                    t=DMA_T, ch=2),
                t_encT[tg * DMA_T:(tg + 1) * DMA_T].rearrange(
                    "t (ch p) hw -> p t ch hw", p=128))
            o_sb = outsp.tile([128, 2048 * DMA_T], bf16, tag="o")

            if MODE == "dma_only":
                nc.scalar.dma_start(
                    t_out[tg * DMA_T:(tg + 1) * DMA_T].rearrange(
                        "t (fh p) hw -> p t fh hw", p=128),
                    enc_sb[:].rearrange("p (t fh hw) -> p t fh hw",
                                        t=DMA_T, fh=2))
                continue

            for tl in range(DMA_T):
                ti = tg * DMA_T + tl
                eb = tl * 2048
                ob = tl * 2048

                # ---- M_t: pm[gm*32+c, ch*256+f] = M[ch*128+gm*32+c, f]
                pm = ps_m.tile([128, 512], f32, tag="pm")
                for ch in range(2):
                    for gm in range(4):
                        g = ch * 4 + gm
                        nc.tensor.matmul(
                            pm[gm * 32:(gm + 1) * 32,
                               ch * 256:(ch + 1) * 256],
                            s_dq[:, ti * C + g * CG: ti * C + (g + 1) * CG],
                            s_wp[:, g * F:(g + 1) * F],
                            tile_position=(0, gm * 32))
                m_sb = mp.tile([128, 512], bf16, tag="m")
                if M_EVAC == "act":
                    nc.scalar.copy(m_sb[:], pm[:])
                else:
                    nc.vector.tensor_copy(m_sb[:], pm[:])

                # ---- outT: po[f', hw'] = sum_C M[C, f']*encT[C, hw']
                for fh in range(2):
                    for hwh in range(2):
                        po = ps_o.tile([128, 512], f32, tag="po")
                        for ch in range(2):
                            nc.tensor.matmul(
                                po[:],
                                m_sb[:, ch * 256 + fh * 128:
                                     ch * 256 + (fh + 1) * 128],
                                enc_sb[:, eb + ch * 1024 + hwh * 512:
                                       eb + ch * 1024 + (hwh + 1) * 512],
                                start=(ch == 0),
                                stop=(ch == 1 and not with_bias))
                        if with_bias:
                            nc.tensor.matmul(
                                po[:], s_bias[0:1, fh * 128:(fh + 1) * 128],
                                s_ones[0:1, :], start=False, stop=True,
                                skip_group_check=True)
                        dst = o_sb[:, ob + fh * 1024 + hwh * 512:
                                   ob + fh * 1024 + (hwh + 1) * 512]
                        use_act = (EVAC == "act"
                                   or (EVAC == "split" and evac_k % 2 == 0))
                        if use_act:
                            nc.scalar.activation(dst, po[:], Relu)
                        else:
                            nc.vector.tensor_scalar_max(dst, po[:], 0.0)
                        evac_k += 1

            nc.scalar.dma_start(
                t_out[tg * DMA_T:(tg + 1) * DMA_T].rearrange(
                    "t (fh p) hw -> p t fh hw", p=128),
                o_sb[:].rearrange("p (t fh hw) -> p t fh hw",
                                  t=DMA_T, fh=2))

    nc.compile()
    return nc


def _prep_inputs(btn_dec, btn_enc, W, bias, with_bias):
    """Host-side layout prep: bf16 cast + enc transpose. Returns per-core
    input maps for run_bass_kernel_spmd."""
    import ml_dtypes
    bf16 = ml_dtypes.bfloat16

    enc_bf = np.asarray(btn_enc, dtype=np.float32).reshape(B, T, HW, C) \
        .astype(bf16)
    encT = np.ascontiguousarray(enc_bf.transpose(0, 1, 3, 2))  # [B,T,C,HW]
    dq = np.asarray(btn_dec, dtype=np.float32).reshape(B, Q, T * C) \
        .astype(bf16)
    wp = np.ascontiguousarray(
        np.asarray(W, dtype=np.float32).reshape(Q, G * F)).astype(bf16)
    in_maps = []
    for i in range(B):
        m = {"encT": encT[i], "dq": dq[i], "wp": wp}
        if with_bias:
            m["biasT"] = np.asarray(bias, dtype=np.float32) \
                .reshape(1, F).astype(bf16)
        in_maps.append(m)
    return in_maps


def kernel(btn_dec, btn_enc, W, bias):
    from concourse.bass_utils import run_bass_kernel_spmd

    bias = np.asarray(bias, dtype=np.float32)
    with_bias = bool(np.any(bias))
    key = ("nc", with_bias)
    if key not in _cache:
        _cache[key] = _build(with_bias)
    nc = _cache[key]

    in_maps = _prep_inputs(btn_dec, btn_enc, W, bias, with_bias)
    res = run_bass_kernel_spmd(nc, in_maps, core_ids=list(range(B)))
    outT = np.stack([np.asarray(res.results[i]["outT"]) for i in range(B)])
    out = outT.astype(np.float32).transpose(0, 1, 3, 2)  # [B,T,HW,F]
    return np.ascontiguousarray(out.reshape(B, T, 32, 32, F))


# revision 11
# speedup vs baseline: 1.2080x; 1.2080x over previous
"""Trainium2 Bass kernel for nn_AttnMap: out = relu(einsum(dec,enc) @ W + bias).

Math: scores[b,t,hw,(q,g)] = sum_c dec[b,g,q,t,c] * enc[b,t,hw,(g,c)]
      out = relu(scores @ W + bias)
Fusion: out[b,t] = relu(enc[b,t] @ M_t + bias) with
      M_t[(g,c), f] = sum_q dec[b,q,t,(g,c)] * W[q*8+g, f]   ([256,256] per t)

Sharding: data-parallel over batch b across the 8 NeuronCores.

Device kernel (per core, per t = 0..15), all-bf16 data path:
  enc arrives HOST-prepared as a partition-major SBUF image
  encP[p, (t, ch, hw)] = enc[t, hw, ch*128+p] (bf16), and the output is
  written as the analogous image outP[p, (t, fh, hw)] = out[t, hw, fh*128+p]
  (un-shuffled on host).  The device therefore does zero transposes, half
  the f32 HBM traffic, and every DMA line is one contiguous 4*DMA_T KiB run.
  1. DMA encP t-chunks [128, 2048*DMA_T] bf16 -> SBUF.
  2. M_t via 8 bf16 matmuls (K=16 over q, col-group-tiled) -> PSUM
     -> bf16 SBUF (DVE/ACT).
  3. outT chunks po[f128, hw512] via 2 accumulating bf16 matmuls
     (M_t chunk stationary [C128, f128], encT moving [C128, hw512])
     -> relu-evac (ACT/DVE) -> bf16 SBUF -> DMA out.
"""
import numpy as np
from contextlib import ExitStack

B, T, HW, C, F = 8, 16, 1024, 256, 256
G, CG, Q = 8, 32, 16  # heads, head dim, queries

_cache = {}


def _build(with_bias: bool, reps: int = 1, tune: dict | None = None):
    import concourse.tile as tile
    from concourse import bacc, mybir

    tune = dict(tune or {})
    DMA_T = tune.get("dma_t", 2)        # t's per enc/out DMA
    BUFS_ENC = tune.get("bufs_enc", 4)
    BUFS_OUT = tune.get("bufs_out", 4)
    BUFS_PO = tune.get("bufs_po", 3)    # fat [128,1024] tiles (2 banks each)
    BUFS_PM = tune.get("bufs_pm", 2)
    EVAC = tune.get("evac", "act")      # po evac: act|dve|split
    M_EVAC = tune.get("m_evac", "dve")  # pm evac: dve|act|alt
    FAT = tune.get("fat", True)         # 2-bank po tiles, 1 evac per fh
    CONST_ENG = tune.get("const_eng", "gpsimd")
    MODE = tune.get("mode", "full")     # full|dma_only|dma_only_1

    f32 = mybir.dt.float32
    bf16 = mybir.dt.bfloat16
    Relu = mybir.ActivationFunctionType.Relu

    nc = bacc.Bacc("TRN2", target_bir_lowering=False, debug=False,
                   num_devices=8)

    t_encP = nc.dram_tensor("encP", [128, T * 2048], bf16,
                            kind="ExternalInput").ap()
    t_dq = nc.dram_tensor("dq", [Q, T * C], bf16, kind="ExternalInput").ap()
    t_wp = nc.dram_tensor("wp", [Q, G * F], bf16, kind="ExternalInput").ap()
    if with_bias:
        t_bias = nc.dram_tensor("biasT", [1, F], bf16,
                                kind="ExternalInput").ap()
    t_outP = nc.dram_tensor("outP", [128, T * 2048], bf16,
                            kind="ExternalOutput").ap()

    with tile.TileContext(nc) as tc, ExitStack() as ctx:
        const = ctx.enter_context(tc.tile_pool(name="const", bufs=1))
        encp = ctx.enter_context(tc.tile_pool(name="encp", bufs=BUFS_ENC))
        outsp = ctx.enter_context(tc.tile_pool(name="outsp", bufs=BUFS_OUT))
        mp = ctx.enter_context(tc.tile_pool(name="mp", bufs=2))
        ps_m = ctx.enter_context(tc.tile_pool(name="ps_m", bufs=BUFS_PM,
                                              space="PSUM"))
        ps_o = ctx.enter_context(tc.tile_pool(name="ps_o", bufs=BUFS_PO,
                                              space="PSUM"))

        cin = nc.gpsimd if CONST_ENG == "gpsimd" else nc.sync
        # dec as [q, (t, g, c)] bf16, W permuted on host to [q, (g, f)] bf16
        s_dq = const.tile([Q, T * C], bf16, tag="dq")
        cin.dma_start(s_dq[:], t_dq)
        s_wp = const.tile([Q, G * F], bf16, tag="wp")
        cin.dma_start(s_wp[:], t_wp)
        if with_bias:
            s_ones = const.tile([1, 512], bf16, tag="ones")
            nc.gpsimd.memset(s_ones[:], 1.0)
            s_bias = const.tile([1, F], bf16, tag="bias")
            cin.dma_start(s_bias[:], t_bias)

        rep_loop = (tc.For_i(0, reps, 1,
                             hint_engines=(mybir.EngineType.PE,
                                           mybir.EngineType.DVE,
                                           mybir.EngineType.Activation,
                                           mybir.EngineType.SP),
                             staggered_reset=tune.get("stag", True))
                    if reps > 1 else None)
        if rep_loop is not None:
            ctx.enter_context(rep_loop)

        XRING = tune.get("xring", False)   # alternate in/out across rings
        SWIN = tune.get("swin", False)     # enc loads via SWDGE (gpsimd)
        evac_k = 0
        for tg in range(T // DMA_T):
            cb = tg * DMA_T * 2048  # dram col base
            # encP load: sbuf[p, (tl, ch, hw)] = enc[t, hw, ch*128+p]
            enc_sb = encp.tile([128, 2048 * DMA_T], bf16, tag="enc")
            in_eng = (nc.gpsimd if SWIN
                      else (nc.scalar if (XRING and tg % 2) else nc.sync))
            OUT_ENG = tune.get("out_eng", "scalar")  # scalar|gpsimd|alt
            if OUT_ENG == "gpsimd":
                out_eng = nc.gpsimd
            elif OUT_ENG == "alt":
                out_eng = nc.gpsimd if tg % 2 else nc.scalar
            else:
                out_eng = nc.sync if (XRING and tg % 2) else nc.scalar
            in_eng.dma_start(enc_sb[:],
                             t_encP[:, cb: cb + 2048 * DMA_T])
            o_sb = outsp.tile([128, 2048 * DMA_T], bf16, tag="o")

            if MODE == "dma_only":
                out_eng.dma_start(t_outP[:, cb: cb + 2048 * DMA_T],
                                  enc_sb[:])
                continue

            for tl in range(DMA_T):
                ti = tg * DMA_T + tl
                eb = tl * 2048
                ob = tl * 2048

                # ---- M_t: pm[gm*32+c, ch*256+f] = M[ch*128+gm*32+c, f]
                pm = ps_m.tile([128, 512], f32, tag="pm")
                for ch in range(2):
                    for gm in range(4):
                        g = ch * 4 + gm
                        nc.tensor.matmul(
                            pm[gm * 32:(gm + 1) * 32,
                               ch * 256:(ch + 1) * 256],
                            s_dq[:, ti * C + g * CG: ti * C + (g + 1) * CG],
                            s_wp[:, g * F:(g + 1) * F],
                            tile_position=(0, gm * 32))
                m_sb = mp.tile([128, 512], bf16, tag="m")
                m_on_act = (M_EVAC == "act"
                            or (M_EVAC == "alt" and ti % 2 == 0))
                if m_on_act:
                    nc.scalar.copy(m_sb[:], pm[:])
                else:
                    nc.vector.tensor_copy(m_sb[:], pm[:])

                # ---- outT: po[f', hw'] = sum_C M[C, f']*encT[C, hw']
                for fh in range(2):
                    po = ps_o.tile([128, 1024 if FAT else 512], f32,
                                   tag="po")
                    for hwh in range(2):
                        pslice = (po[:, hwh * 512:(hwh + 1) * 512]
                                  if FAT else po[:])
                        for ch in range(2):
                            nc.tensor.matmul(
                                pslice,
                                m_sb[:, ch * 256 + fh * 128:
                                     ch * 256 + (fh + 1) * 128],
                                enc_sb[:, eb + ch * 1024 + hwh * 512:
                                       eb + ch * 1024 + (hwh + 1) * 512],
                                start=(ch == 0),
                                stop=(ch == 1 and not with_bias))
                        if with_bias:
                            nc.tensor.matmul(
                                pslice, s_bias[0:1, fh * 128:(fh + 1) * 128],
                                s_ones[0:1, :], start=False, stop=True,
                                skip_group_check=True)
                        if not FAT:
                            dst = o_sb[:, ob + fh * 1024 + hwh * 512:
                                       ob + fh * 1024 + (hwh + 1) * 512]
                            use_act = (EVAC == "act" or
                                       (EVAC == "split" and evac_k % 2 == 0))
                            if use_act:
                                nc.scalar.activation(dst, po[:], Relu)
                            else:
                                nc.vector.tensor_scalar_max(dst, po[:], 0.0)
                            evac_k += 1
                    if FAT:
                        dst = o_sb[:, ob + fh * 1024: ob + (fh + 1) * 1024]
                        use_act = (EVAC == "act" or
                                   (EVAC == "split" and evac_k % 2 == 0) or
                                   (EVAC == "alt32"
                                    and not (ti % 2 == 1 and fh == 1)))
                        if use_act:
                            nc.scalar.activation(dst, po[:], Relu)
                        else:
                            nc.vector.tensor_scalar_max(dst, po[:], 0.0)
                        evac_k += 1
                if tune.get("osplit", False):
                    out_eng.dma_start(
                        t_outP[:, cb + tl * 2048: cb + (tl + 1) * 2048],
                        o_sb[:, tl * 2048:(tl + 1) * 2048])

            if not tune.get("osplit", False):
                out_eng.dma_start(t_outP[:, cb: cb + 2048 * DMA_T], o_sb[:])

    nc.compile()
    return nc


def _prep_inputs(btn_dec, btn_enc, W, bias, with_bias):
    """Host-side layout prep: bf16 cast + partition-major enc image.
    Returns per-core input maps for run_bass_kernel_spmd."""
    import ml_dtypes
    bf16 = ml_dtypes.bfloat16

    enc_bf = np.asarray(btn_enc, dtype=np.float32).reshape(B, T, HW, C) \
        .astype(bf16)
    # encP[b, p, t, ch, hw] = enc[b, t, hw, ch*128 + p]
    encP = np.ascontiguousarray(
        enc_bf.reshape(B, T, HW, 2, 128).transpose(0, 4, 1, 3, 2)
    ).reshape(B, 128, T * 2048)
    dq = np.asarray(btn_dec, dtype=np.float32).reshape(B, Q, T * C) \
        .astype(bf16)
    wp = np.ascontiguousarray(
        np.asarray(W, dtype=np.float32).reshape(Q, G * F)).astype(bf16)
    in_maps = []
    for i in range(B):
        m = {"encP": encP[i], "dq": dq[i], "wp": wp}
        if with_bias:
            m["biasT"] = np.asarray(bias, dtype=np.float32) \
                .reshape(1, F).astype(bf16)
        in_maps.append(m)
    return in_maps


def kernel(btn_dec, btn_enc, W, bias):
    from concourse.bass_utils import run_bass_kernel_spmd

    bias = np.asarray(bias, dtype=np.float32)
    with_bias = bool(np.any(bias))
    key = ("nc", with_bias)
    if key not in _cache:
        _cache[key] = _build(with_bias)
    nc = _cache[key]

    in_maps = _prep_inputs(btn_dec, btn_enc, W, bias, with_bias)
    res = run_bass_kernel_spmd(nc, in_maps, core_ids=list(range(B)))
    outP = np.stack([np.asarray(res.results[i]["outP"]) for i in range(B)])
    # outP[b, p, t, fh, hw] -> out[b, t, hw, fh*128+p]
    out = outP.reshape(B, 128, T, 2, HW).transpose(0, 2, 4, 3, 1) \
        .astype(np.float32)
    return np.ascontiguousarray(out.reshape(B, T, 32, 32, F))


# revision 13
# speedup vs baseline: 1.3231x; 1.0953x over previous
"""Trainium2 Bass kernel for nn_AttnMap: out = relu(einsum(dec,enc) @ W + bias).

Math: scores[b,t,hw,(q,g)] = sum_c dec[b,g,q,t,c] * enc[b,t,hw,(g,c)]
      out = relu(scores @ W + bias)
Fusion: out[b,t] = relu(enc[b,t] @ M_t + bias) with
      M_t[(g,c), f] = sum_q dec[b,q,t,(g,c)] * W[q*8+g, f]   ([256,256] per t)

Sharding: data-parallel over batch b across the 8 NeuronCores.

Device kernel (per core, per t = 0..15), all-bf16 data path:
  enc arrives HOST-prepared as a partition-major SBUF image
  encP[p, (t, ch, hw)] = enc[t, hw, ch*128+p] (bf16), and the output is
  written as the analogous image outP[p, (t, fh, hw)] = out[t, hw, fh*128+p]
  (un-shuffled on host).  The device therefore does zero transposes, half
  the f32 HBM traffic, and every DMA line is one contiguous 4*DMA_T KiB run.
  1. DMA encP t-chunks [128, 2048*DMA_T] bf16 -> SBUF.
  2. M_t via 8 bf16 matmuls (K=16 over q, col-group-tiled) -> PSUM
     -> bf16 SBUF (DVE/ACT).
  3. outT chunks po[f128, hw512] via 2 accumulating bf16 matmuls
     (M_t chunk stationary [C128, f128], encT moving [C128, hw512])
     -> relu-evac (ACT/DVE) -> bf16 SBUF -> DMA out.
"""
import numpy as np
from contextlib import ExitStack

B, T, HW, C, F = 8, 16, 1024, 256, 256
G, CG, Q = 8, 32, 16  # heads, head dim, queries

_cache = {}


def _build(with_bias: bool, reps: int = 1, tune: dict | None = None):
    import concourse.tile as tile
    from concourse import bacc, mybir

    tune = dict(tune or {})
    DMA_T = tune.get("dma_t", 2)        # t's per enc/out DMA
    BUFS_ENC = tune.get("bufs_enc", 4)
    BUFS_OUT = tune.get("bufs_out", 4)
    BUFS_PO = tune.get("bufs_po", 3)    # fat [128,1024] tiles (2 banks each)
    BUFS_PM = tune.get("bufs_pm", 2)
    EVAC = tune.get("evac", "act")      # po evac: act|dve|split
    M_EVAC = tune.get("m_evac", "dve")  # pm evac: dve|act|alt
    FAT = tune.get("fat", True)         # 2-bank po tiles, 1 evac per fh
    CONST_ENG = tune.get("const_eng", "gpsimd")
    MODE = tune.get("mode", "full")     # full|dma_only|dma_only_1

    f32 = mybir.dt.float32
    bf16 = mybir.dt.bfloat16
    Relu = mybir.ActivationFunctionType.Relu

    nc = bacc.Bacc("TRN2", target_bir_lowering=False, debug=False,
                   num_devices=8)

    t_encP = nc.dram_tensor("encP", [128, T * 2048], bf16,
                            kind="ExternalInput").ap()
    t_dq = nc.dram_tensor("dq", [Q, T * C], bf16, kind="ExternalInput").ap()
    t_wp = nc.dram_tensor("wp", [Q, G * F], bf16, kind="ExternalInput").ap()
    if with_bias:
        t_bias = nc.dram_tensor("biasT", [1, F], bf16,
                                kind="ExternalInput").ap()
    t_outP = nc.dram_tensor("outP", [128, T * 2048], bf16,
                            kind="ExternalOutput").ap()

    with tile.TileContext(nc) as tc, ExitStack() as ctx:
        const = ctx.enter_context(tc.tile_pool(name="const", bufs=1))
        encp = ctx.enter_context(tc.tile_pool(name="encp", bufs=BUFS_ENC))
        outsp = ctx.enter_context(tc.tile_pool(name="outsp", bufs=BUFS_OUT))
        mp = ctx.enter_context(tc.tile_pool(name="mp", bufs=2))
        ps_m = ctx.enter_context(tc.tile_pool(name="ps_m", bufs=BUFS_PM,
                                              space="PSUM"))
        ps_o = ctx.enter_context(tc.tile_pool(name="ps_o", bufs=BUFS_PO,
                                              space="PSUM"))

        cin = nc.gpsimd if CONST_ENG == "gpsimd" else nc.sync
        # dec as [q, (t, g, c)] bf16, W permuted on host to [q, (g, f)] bf16
        s_dq = const.tile([Q, T * C], bf16, tag="dq")
        cin.dma_start(s_dq[:], t_dq)
        s_wp = const.tile([Q, G * F], bf16, tag="wp")
        cin.dma_start(s_wp[:], t_wp)
        if with_bias:
            s_ones = const.tile([1, 512], bf16, tag="ones")
            nc.gpsimd.memset(s_ones[:], 1.0)
            s_bias = const.tile([1, F], bf16, tag="bias")
            cin.dma_start(s_bias[:], t_bias)

        rep_loop = (tc.For_i(0, reps, 1,
                             hint_engines=(mybir.EngineType.PE,
                                           mybir.EngineType.DVE,
                                           mybir.EngineType.Activation,
                                           mybir.EngineType.SP),
                             staggered_reset=tune.get("stag", True))
                    if reps > 1 else None)
        if rep_loop is not None:
            ctx.enter_context(rep_loop)

        XRING = tune.get("xring", False)   # alternate in/out across rings
        SWIN = tune.get("swin", False)     # enc loads via SWDGE (gpsimd)
        IG = tune.get("in_group", 1)       # tgs per enc DMA (transfer size)
        OG = tune.get("out_group", 1)      # tgs per out DMA
        evac_k = 0
        enc_sb = o_sb = None
        for tg in range(T // DMA_T):
            cb = tg * DMA_T * 2048  # dram col base
            in_eng = (nc.gpsimd if SWIN
                      else (nc.scalar if (XRING and tg % 2) else nc.sync))
            OUT_ENG = tune.get("out_eng", "scalar")  # scalar|gpsimd|alt
            if OUT_ENG == "gpsimd":
                out_eng = nc.gpsimd
            elif OUT_ENG == "alt":
                out_eng = nc.gpsimd if tg % 2 else nc.scalar
            else:
                out_eng = nc.sync if (XRING and tg % 2) else nc.scalar
            if tg % IG == 0:
                # encP load: sbuf[p, (tl, ch, hw)] = enc[t, hw, ch*128+p]
                enc_sb = encp.tile([128, 2048 * DMA_T * IG], bf16,
                                   tag="enc")
                in_eng.dma_start(enc_sb[:],
                                 t_encP[:, cb: cb + 2048 * DMA_T * IG])
            if tg % OG == 0:
                o_sb = outsp.tile([128, 2048 * DMA_T * OG], bf16, tag="o")

            if MODE == "dma_only":
                if tg % OG == OG - 1:
                    ocb = (tg - (OG - 1)) * DMA_T * 2048
                    out_eng.dma_start(
                        t_outP[:, ocb: ocb + 2048 * DMA_T * OG],
                        enc_sb[:, :2048 * DMA_T * OG])
                continue

            for tl in range(DMA_T):
                ti = tg * DMA_T + tl
                eb = (tg % IG) * DMA_T * 2048 + tl * 2048
                ob = (tg % OG) * DMA_T * 2048 + tl * 2048

                # ---- M_t: pm[gm*32+c, ch*256+f] = M[ch*128+gm*32+c, f]
                pm = ps_m.tile([128, 512], f32, tag="pm")
                for ch in range(2):
                    for gm in range(4):
                        g = ch * 4 + gm
                        nc.tensor.matmul(
                            pm[gm * 32:(gm + 1) * 32,
                               ch * 256:(ch + 1) * 256],
                            s_dq[:, ti * C + g * CG: ti * C + (g + 1) * CG],
                            s_wp[:, g * F:(g + 1) * F],
                            tile_position=(0, gm * 32))
                m_sb = mp.tile([128, 512], bf16, tag="m")
                m_on_act = (M_EVAC == "act"
                            or (M_EVAC == "alt" and ti % 2 == 0))
                if m_on_act:
                    nc.scalar.copy(m_sb[:], pm[:])
                else:
                    nc.vector.tensor_copy(m_sb[:], pm[:])

                # ---- outT: po[f', hw'] = sum_C M[C, f']*encT[C, hw']
                for fh in range(2):
                    po = ps_o.tile([128, 1024 if FAT else 512], f32,
                                   tag="po")
                    for hwh in range(2):
                        pslice = (po[:, hwh * 512:(hwh + 1) * 512]
                                  if FAT else po[:])
                        for ch in range(2):
                            nc.tensor.matmul(
                                pslice,
                                m_sb[:, ch * 256 + fh * 128:
                                     ch * 256 + (fh + 1) * 128],
                                enc_sb[:, eb + ch * 1024 + hwh * 512:
                                       eb + ch * 1024 + (hwh + 1) * 512],
                                start=(ch == 0),
                                stop=(ch == 1 and not with_bias))
                        if with_bias:
                            nc.tensor.matmul(
                                pslice, s_bias[0:1, fh * 128:(fh + 1) * 128],
                                s_ones[0:1, :], start=False, stop=True,
                                skip_group_check=True)
                        if not FAT:
                            dst = o_sb[:, ob + fh * 1024 + hwh * 512:
                                       ob + fh * 1024 + (hwh + 1) * 512]
                            use_act = (EVAC == "act" or
                                       (EVAC == "split" and evac_k % 2 == 0))
                            if use_act:
                                nc.scalar.activation(dst, po[:], Relu)
                            else:
                                nc.vector.tensor_scalar_max(dst, po[:], 0.0)
                            evac_k += 1
                    if FAT:
                        dst = o_sb[:, ob + fh * 1024: ob + (fh + 1) * 1024]
                        use_act = (EVAC == "act" or
                                   (EVAC == "split" and evac_k % 2 == 0) or
                                   (EVAC == "alt32"
                                    and not (ti % 2 == 1 and fh == 1)))
                        if use_act:
                            nc.scalar.activation(dst, po[:], Relu)
                        else:
                            nc.vector.tensor_scalar_max(dst, po[:], 0.0)
                        evac_k += 1
            if tg % OG == OG - 1:
                ocb = (tg - (OG - 1)) * DMA_T * 2048
                out_eng.dma_start(t_outP[:, ocb: ocb + 2048 * DMA_T * OG],
                                  o_sb[:])

    nc.compile()
    return nc


def _prep_inputs(btn_dec, btn_enc, W, bias, with_bias):
    """Host-side layout prep: bf16 cast + partition-major enc image.
    Returns per-core input maps for run_bass_kernel_spmd."""
    import ml_dtypes
    bf16 = ml_dtypes.bfloat16

    enc_bf = np.asarray(btn_enc, dtype=np.float32).reshape(B, T, HW, C) \
        .astype(bf16)
    # encP[b, p, t, ch, hw] = enc[b, t, hw, ch*128 + p]
    encP = np.ascontiguousarray(
        enc_bf.reshape(B, T, HW, 2, 128).transpose(0, 4, 1, 3, 2)
    ).reshape(B, 128, T * 2048)
    dq = np.asarray(btn_dec, dtype=np.float32).reshape(B, Q, T * C) \
        .astype(bf16)
    wp = np.ascontiguousarray(
        np.asarray(W, dtype=np.float32).reshape(Q, G * F)).astype(bf16)
    in_maps = []
    for i in range(B):
        m = {"encP": encP[i], "dq": dq[i], "wp": wp}
        if with_bias:
            m["biasT"] = np.asarray(bias, dtype=np.float32) \
                .reshape(1, F).astype(bf16)
        in_maps.append(m)
    return in_maps


def kernel(btn_dec, btn_enc, W, bias):
    from concourse.bass_utils import run_bass_kernel_spmd

    bias = np.asarray(bias, dtype=np.float32)
    with_bias = bool(np.any(bias))
    key = ("nc", with_bias)
    if key not in _cache:
        _cache[key] = _build(with_bias)
    nc = _cache[key]

    in_maps = _prep_inputs(btn_dec, btn_enc, W, bias, with_bias)
    res = run_bass_kernel_spmd(nc, in_maps, core_ids=list(range(B)))
    outP = np.stack([np.asarray(res.results[i]["outP"]) for i in range(B)])
    # outP[b, p, t, fh, hw] -> out[b, t, hw, fh*128+p]
    out = outP.reshape(B, 128, T, 2, HW).transpose(0, 2, 4, 3, 1) \
        .astype(np.float32)
    return np.ascontiguousarray(out.reshape(B, T, 32, 32, F))
